# revision 22
# baseline (speedup 1.0000x reference)
"""Trainium2 Bass kernel for nn_DLGN_VT (deep linearly-gated network w/ value tensor).

Math (per batch row b):
    g_i = sigmoid(30 * x @ W_i.T)            i = 1,2,3    [B, 32] each
    out[b] = sum_{ijk} g1[b,i] g2[b,j] g3[b,k] V[i,j,k]

Distribution: pure data-parallel over the batch axis, 8 NeuronCores,
512 rows per core. W_i and V are tiny and replicated.

v2 design (fp16 single-pass, measured rel err ~0.0034 vs 2e-2 budget):
  - fp16 operands everywhere (PE fp16 matmul = same rate as bf16 with 10
    mantissa bits): ONE gating matmul per block replaces the baseline's
    3-pass error-compensated bf16 scheme, and halves the input DMA.
  - xa [128, 704] fp16 = xT shard | W3-replicated-4x | [W1;W2] packed,
    DMA'd on the SP queue; s2 selection tables on the ACT queue in
    parallel; vo = V^T chunks (x256) + ones/256 second on SP.
  - Gating emits g3 logits PRE-REPLICATED across 128 partitions (W3rep
    stationary), so one sigmoid writes e3s[p,b] = g3[p%32,b] straight to
    SBUF: no E3 selection matmul and no PSUM->SBUF copy.
  - A^T[(jk),b] = g2[j,b]*g3[k,b] in 4 pair-blocks: two K=32 selection
    matmuls per pair into a 2-bank PSUM tile ([128,2,512] f32 -- every
    matmul output must be PSUM-bank-aligned, a hard HW rule), then an
    elementwise multiply vs broadcast e3s. Pairs 0-1 multiply on the DVE
    directly from PSUM; pairs 2-3 are staged to SBUF fp16 by the
    otherwise-idle ACT engine so their multiplies run in the DVE's 2x
    all-SBUF-2-byte mode (~600ns vs ~1130ns). (GpSimd cannot touch PSUM,
    so it cannot help here.)
  - PE clock ramp: dummy matmuls bridge every would-be idle gap from
    engine start to the gating matmuls (an idle PE decays to ~2x slower
    p-state; K=128 moving fetches additionally cap at 2 cyc/row).
  - C^T[i,b] accumulates over 8 fp16 matmuls (V^T chunks stationary);
    out[b] = (ones/256)^T @ (g1 .* C') with V pre-scaled by 256 to keep
    the fp16 y tile in normal range.
"""

import numpy as np

import concourse.bass as bass
import concourse.bacc as bacc
import concourse.mybir as mybir
import concourse.tile as tile
from concourse.alu_op_type import AluOpType
from concourse.bass_utils import run_bass_kernel_spmd

NCORES = 8
B, D, N = 4096, 128, 32
BL = B // NCORES  # 512 batch rows per core
BETA = 30.0
NQ = 8   # 128-row blocks of the jk=1024 plane
NP = 4   # pairs of blocks

F32 = mybir.dt.float32
F16 = mybir.dt.float16

VSCALE = 256.0

# xa packed fp16 [128, 704]: xT | W3rep (128 cols) | [W1;W2] (64 cols)
X0, X1 = 0, BL
WA0, WA1 = X1, X1 + 128
WB0, WB1 = WA1, WA1 + 64
XA_C = WB1
# vo [128, 257]: V^T chunks [128, 256] | ones/256 at rows 0:32, col 256
VT0, VT1 = 0, 256
ON0 = 256
VO_C = 257

N_WARMUP = 5   # dummy matmuls to warm the PE HAM clock gate
TTC = 368      # DVE takes cols 0:TTC of each pair multiply, GpSimd the rest


def build_nc():
    # Bacc (not raw Bass): its compile passes split multi-wait sync infos
    # (TRN2 allows at most one sync wait per compute instruction).
    nc = bacc.Bacc(None)
    xa_d = nc.declare_dram_parameter("xa", [128, XA_C], F16, isOutput=False)
    vo_d = nc.declare_dram_parameter("vo", [128, VO_C], F16, isOutput=False)
    s2_d = nc.declare_dram_parameter("s2", [32, NQ * 128], F16, isOutput=False)
    out_d = nc.declare_dram_parameter("out", [1, BL], F32, isOutput=True)

    sig = mybir.ActivationFunctionType.Sigmoid

    with tile.TileContext(nc) as tc:
        with (
            tc.tile_pool(name="const", bufs=1) as cpool,
            tc.tile_pool(name="work", bufs=1) as wpool,
            tc.tile_pool(name="atp", bufs=1) as apool,
            tc.tile_pool(name="psA", bufs=1, space="PSUM") as psA,
            tc.tile_pool(name="psB", bufs=3, space="PSUM") as psB,
        ):
            xa = cpool.tile([128, XA_C], F16)
            vo = cpool.tile([128, VO_C], F16)
            s2t = cpool.tile([64, NQ * 128], F16)  # rows 32:64 hold S2
            nc.sync.dma_start(xa[:], xa_d[:])
            nc.scalar.dma_start(s2t[32:64, :], s2_d[:])
            nc.sync.dma_start(vo[:], vo_d[:])

            xt = xa[:, X0:X1]
            wa = xa[:, WA0:WA1]
            wb = xa[:, WB0:WB1]
            vts = vo[:, VT0:VT1]
            ones = vo[0:32, ON0 : ON0 + 1]

            # ---- PE warmup into the E3 PSUM bank (overwritten later by the
            # real gating matmul's start=True). memset on GpSimd: its queue
            # is free earliest. ----
            e3ps = psA.tile([128, BL], F32, tag="ps")
            g21ps = psA.tile([64, BL], F32, tag="ps2")
            wz = wpool.tile([128, BL], F16)
            nc.gpsimd.memset(wz[:], 0.0)
            for _ in range(N_WARMUP):
                nc.tensor.matmul(e3ps[:], wz[:, 0:128], wz[:],
                                 start=True, stop=True)
            # short tail warmup: bridges the last ~200ns until the input
            # DMA semaphore fires, so the gating matmul runs at full clock
            nc.tensor.matmul(e3ps[:, 0:128], wz[:, 0:128], wz[:, 0:128],
                             start=True, stop=True)

            # ---- gating: one fp16 matmul per block. e3 first, and the
            # sigmoids in the same order: every E2 selection matmul then
            # waits on sig-g21, whose ACT-engine completion transitively
            # implies e3s is ready, so each A^T multiply needs only its E2
            # matmul's semaphore -- the scheduler would otherwise serialize
            # the DVE and GpSimd multiplies against each other to satisfy
            # their two-producer dependency with a single wait. ----
            nc.tensor.matmul(g21ps[:], wb, xt, start=True, stop=True)
            nc.tensor.matmul(e3ps[:], wa, xt, start=True, stop=True)

            g21 = wpool.tile([64, BL], F16)   # g1 rows 0:32, g2 rows 32:64
            e3s = wpool.tile([128, BL], F16)  # e3s[p,b] = g3[p%32, b]
            nc.scalar.activation(g21[:], g21ps[:], sig, scale=BETA)
            nc.scalar.activation(e3s[:], e3ps[:], sig, scale=BETA)
            g1t = g21[0:32, :]
            g2t = g21[32:64, :]  # base partition 32, matching s2t rows

            # ---- A^T pair-blocks: E2 selection matmuls into 2-bank PSUM
            # pair tiles ([128, 2, 512] f32: each h-half is exactly one
            # PSUM bank, keeping every matmul output bank-aligned), then
            # one DVE multiply per pair against broadcast e3s ----
            ats = []
            for p in range(NP):
                e2ps = psB.tile([128, 2, BL], F32, tag="e2")
                if p == 0:
                    nc.tensor.matmul(e2ps[:, 0, :], wz[:, 0:128], wz[:],
                                     start=True, stop=True)
                for h in range(2):
                    q = 2 * p + h
                    s2q = s2t[32:64, 128 * q : 128 * (q + 1)]
                    nc.tensor.matmul(
                        e2ps[:, h, :], s2q, g2t, start=True, stop=True)
                at = apool.tile([128, 2, BL], F16, tag=f"at_{p}")
                e3a = e3s[:].unsqueeze(1).broadcast_to((128, 2, BL))
                if p < 2:
                    nc.vector.tensor_tensor(at[:], e2ps[:], e3a, AluOpType.mult)
                else:
                    # ACT (idle after the sigmoids) stages the pair into SBUF
                    # fp16; the all-2-byte SBUF multiply then runs at 2x on DVE
                    sbg = wpool.tile([128, 2, BL], F16, tag=f"sbg_{p}")
                    nc.scalar.copy(sbg[:], e2ps[:])
                    nc.vector.tensor_tensor(at[:], sbg[:], e3a, AluOpType.mult)
                ats.append(at)


            # ---- C accumulation over the 8 blocks, split into b-column
            # halves with separate PSUM banks (the two psA ring tags), so
            # the left half's y/ones/copy tail pipelines against the right
            # half's final C matmuls. Every matmul output stays at a tile
            # start and each accumulation group owns a whole bank. ----
            HB = BL // 2
            cpsL = psA.tile([N, HB], F32, tag="ps")
            cpsR = psA.tile([N, HB], F32, tag="ps2")
            for q in range(NQ):
                p, h = q // 2, q % 2
                vq = vts[:, 32 * q : 32 * (q + 1)]
                nc.tensor.matmul(
                    cpsL[:], vq, ats[p][:, h, 0:HB],
                    start=(q == 0), stop=(q == NQ - 1),
                )
                nc.tensor.matmul(
                    cpsR[:], vq, ats[p][:, h, HB:BL],
                    start=(q == 0), stop=(q == NQ - 1),
                )

            # ---- out = (ones/256).T @ (g1t .* C'), pipelined halves ----
            yL = wpool.tile([N, HB], F16)
            yR = wpool.tile([N, HB], F16)
            nc.vector.tensor_tensor(yL[:], cpsL[:], g1t[:, 0:HB], AluOpType.mult)
            nc.vector.tensor_tensor(yR[:], cpsR[:], g1t[:, HB:BL], AluOpType.mult)
            opsL = psA.tile([1, HB], F32, tag="ps")
            opsR = psA.tile([1, HB], F32, tag="ps2")
            nc.tensor.matmul(opsL[:], ones, yL[:], start=True, stop=True)
            nc.tensor.matmul(opsR[:], ones, yR[:], start=True, stop=True)
            outsL = wpool.tile([1, HB], F32)
            outsR = wpool.tile([1, HB], F32)
            nc.vector.tensor_copy(outsL[:], opsL[:])
            nc.vector.tensor_copy(outsR[:], opsR[:])
            # two output DMAs on the two hardware DGE queues: their
            # trigger/completion chains overlap instead of serializing
            nc.sync.dma_start(out_d[:, 0:HB], outsL[:])
            nc.scalar.dma_start(out_d[:, HB:BL], outsR[:])

    nc.finalize()
    return nc


def host_prep(x, W1, W2, W3, V):
    """Build per-core input maps (all numpy, fp32 in / packed fp16 out)."""
    x = np.asarray(x, dtype=np.float32)
    W1 = np.asarray(W1, dtype=np.float32)
    W2 = np.asarray(W2, dtype=np.float32)
    W3 = np.asarray(W3, dtype=np.float32)
    V = np.asarray(V, dtype=np.float32)

    xT = np.ascontiguousarray(x.T).astype(np.float16)  # [128, 4096]

    # wa[d, p] = W3[p%32, d]: gating emits g3 logits already replicated in
    # the e3 pattern. wb = [W1; W2]^T so g1 lands at partition base 0 (for
    # the final y multiply) and g2 at base 32 (matching the s2 tables).
    wa = W3.T[:, np.arange(128) % 32].astype(np.float16)  # [128, 128]
    wb = np.concatenate([W1, W2], axis=0).T.astype(np.float16)  # [128, 64]

    # V^T chunks (x256): vts[p, 32q + i] = 256 * V[0, i, jk] at jk = 128q + p
    Vr = (V * VSCALE).reshape(N, N * N)
    VT = np.ascontiguousarray(Vr.T)  # [jk, i]
    VTs = VT.reshape(NQ, 128, N).transpose(1, 0, 2).reshape(128, NQ * N)

    # E2 selection: S2[j', q*128 + p] = 1 iff j' == 4q + p//32
    S2 = np.zeros((N, NQ, 128), dtype=np.float16)
    for q in range(NQ):
        for p in range(128):
            S2[4 * q + p // 32, q, p] = 1.0
    S2pack = S2.reshape(N, NQ * 128)

    vo = np.zeros((128, VO_C), dtype=np.float16)
    vo[:, VT0:VT1] = VTs.astype(np.float16)
    vo[0:32, ON0] = np.float16(1.0 / VSCALE)

    xa = np.zeros((128, XA_C), dtype=np.float16)
    xa[:, WA0:WA1] = wa
    xa[:, WB0:WB1] = wb

    in_maps = []
    for c in range(NCORES):
        m = xa.copy()
        m[:, X0:X1] = xT[:, c * BL : (c + 1) * BL]
        in_maps.append({"xa": m, "vo": vo, "s2": S2pack})
    return in_maps


_CACHED_NC = None


def _ensure_ntff_hook():
    """The agent image's `antenv` package lacks `axon_hooks`; synthesize it
    and register the boot module's ctypes-based NTFF profile hook so
    run_bass_kernel_spmd(trace=True) can capture neuron-profile output."""
    import sys, types

    try:
        from antenv.axon_hooks import get_axon_ntff_profile_hook  # noqa: F401

        return
    except ImportError:
        pass
    import antenv
    from trn_agent_boot.trn_boot import _ntff_profile_via_ctypes

    mod = types.ModuleType("antenv.axon_hooks")
    mod._hook = _ntff_profile_via_ctypes("/opt/axon/libaxon_pjrt.so")
    mod.get_axon_ntff_profile_hook = lambda: mod._hook
    mod.set_axon_ntff_profile_hook = lambda h: setattr(mod, "_hook", h)
    sys.modules["antenv.axon_hooks"] = mod
    antenv.axon_hooks = mod


def run(inputs, trace=False, **trace_kwargs):
    """Run the kernel on 8 cores. Returns (out [4096] f32, BassKernelResults)."""
    global _CACHED_NC
    if trace:
        _ensure_ntff_hook()
    if _CACHED_NC is None:
        _CACHED_NC = build_nc()
    in_maps = host_prep(
        inputs["x"], inputs["W1"], inputs["W2"], inputs["W3"], inputs["V"]
    )
    res = run_bass_kernel_spmd(
        _CACHED_NC, in_maps, core_ids=list(range(NCORES)), trace=trace, **trace_kwargs
    )
    out = np.concatenate(
        [np.asarray(res.results[c]["out"]).reshape(BL) for c in range(NCORES)]
    ).astype(np.float32)
    return out, res


def kernel(**inputs):
    out, _ = run(inputs, trace=False)
    return out
